# revision 23
# baseline (speedup 1.0000x reference)
"""Causal multi-head attention (B=2, S=2048, H=16, D=64, fp32) on 8 trn2 cores.

Sharding: the 32 (batch, head) attention instances are split 4-per-core
(data parallel over B, tensor parallel over H) -- no collectives needed.

Device kernel (per core): instances are processed in PAIRS packed into the
128-deep PE array (K=64 each, row groups (0,0)/(64,0) stream concurrently).

Per pair, per query chunk of 512 (processed LARGEST-first so the PE's HAM
clock gate warms up early and stays warm):
  - scores transposed: S^T[k, q] via matmul(lhsT=K^T tile, rhs=Q^T chunk),
    both instances into one 2-bank PSUM tile.
  - P^T = exp(sm_scale * S^T):
      * non-diagonal k tiles: split between ScalarE ACTIVATE Exp and
        VectorE Schraudolph (tensor_scalar y = s*A + B written as int16,
        bits read back as fp16 ~ exp).
      * diagonal-region k tiles: VectorE scalar_tensor_tensor fusing the
        Schraudolph exp AND the causal mask in ONE op:
          y = int16((s + B/A) * mtile),  mtile = A where q >= k else 0
        masked lanes produce exactly 0.0 -> int16 0 -> fp16 +0.0.
  - ctx^T[d, q] = sum_k V_ext[k, d] P^T[k, q] accumulated in PSUM per
    instance; V_ext carries a ones column so row 64 is the softmax
    denominator. The [65, S] unnormalized ctx^T goes to HBM as fp16; the
    host divides by the denominator row and transposes into output layout.

Input DMAs are SEGMENTED (512-col chunks of Q^T/K^T, half-tiles of V) and
issued in first-use order so the first score matmul can start ~1.5us in,
instead of waiting for the full 3 MiB input fill. A short warmup matmul
burst covers the initial DMA window to keep the HAM clock gate warming.
"""

import numpy as np

B, S, H, D = 2, 2048, 16, 64
NCORES = 8
NI = (B * H) // NCORES  # attention instances per core
QC = 512  # query-chunk width (one PSUM bank of fp32)
SM_SCALE = 0.125  # 1/sqrt(D)
SCH_A = 1024.0 / np.log(2.0) * SM_SCALE  # Schraudolph multiplier
SCH_B = 15360.0 - 45.0  # fp16 exponent bias + centering constant
# fp16-exact Schraudolph multiplier used in the fused mask tile, and the
# matching pre-add constant so (s + SCH_BA) * SCH_A16 == s*SCH_A16 + SCH_B.
SCH_A16 = float(np.float16(SCH_A))
SCH_BA = SCH_B / SCH_A16
WARM = 14  # upfront warmup matmuls (cover the input-DMA window)
WARM_POST = 4  # extra warmups after the LEAD prefill (exp-latency ramp)

_NC_CACHE = {}


def _build_body(tc, outT, qt, kt, v, mconst, m2, seq, ni):
    import concourse.bass as bass
    from concourse import mybir

    nc = tc.nc
    f32 = mybir.dt.float32
    f16 = mybir.dt.float16
    i16 = mybir.dt.int16
    nkt = seq // 128  # key tiles per instance
    nqc = seq // QC  # query chunks per instance
    kt_per_qc = QC // 128
    assert ni % 2 == 0
    npair = ni // 2

    with (
        tc.tile_pool(name="const", bufs=1) as const_pool,
        tc.tile_pool(name="qk", bufs=npair * nqc) as qk_pool,
        tc.tile_pool(name="vp", bufs=2 * ni) as v_pool,
        tc.tile_pool(name="pt", bufs=6) as pt_pool,
        tc.tile_pool(name="ob", bufs=4) as o_pool,
        tc.tile_pool(name="sps", bufs=3, space="PSUM") as s_psum,
        tc.tile_pool(name="cps", bufs=2, space="PSUM") as c_psum,
    ):
        # --- HAM warmup: keep the PE busy during the first input DMA
        # segments so the clock gate starts warming before real work.
        warm_t = const_pool.tile([128, 256], f16)
        nc.vector.memset(warm_t[:], 0.0)
        for _ in range(WARM):
            wmm = s_psum.tile([128, 2, QC], f32, tag="sc")
            nc.tensor.matmul(
                wmm[:, :, 0:128],
                lhsT=warm_t[:, 0:128],
                rhs=warm_t[:],
                start=True,
                stop=True,
            )

        # --- Input DMAs: coarse (each dma_start costs ~650ns of serial
        # sync-engine issue time), but pair 0's first-needed data is split
        # so the first score matmul can start ~1.3us earlier: k in halves,
        # q's last chunk (the schedule starts at chunk 3) separated.
        kseg = {}  # (pair, half) -> [128, 2*QC] tile (k tiles 8h..8h+7)
        qseg = {}  # (pair, c) -> [128, QC] view into the pair's q tiles
        vseg = {}  # (pair, inst_in_pair) -> [128, nkt, D+1]
        mc = None

        def dma_k(pair, h):
            t = qk_pool.tile([128, 2 * QC], f16, tag="k")
            nc.sync.dma_start(out=t[:], in_=kt[pair][:, 2 * h * QC : 2 * (h + 1) * QC])
            kseg[(pair, h)] = t

        def dma_v(pair, iip):
            t = v_pool.tile([128, nkt, D + 1], f16, tag="v")
            nc.sync.dma_start(out=t[:], in_=v[2 * pair + iip])
            vseg[(pair, iip)] = t

        for pair in range(npair):
            if pair == 0:
                dma_k(pair, 0)
                qhi = qk_pool.tile([128, QC], f16, tag="q")
                nc.sync.dma_start(
                    out=qhi[:], in_=qt[pair][:, (nqc - 1) * QC : nqc * QC]
                )
                dma_k(pair, 1)
                qlo = qk_pool.tile([128, (nqc - 1) * QC], f16, tag="qlo")
                nc.sync.dma_start(out=qlo[:], in_=qt[pair][:, 0 : (nqc - 1) * QC])
                for c in range(nqc - 1):
                    qseg[(pair, c)] = (qlo, c * QC)
                qseg[(pair, nqc - 1)] = (qhi, 0)
                dma_v(pair, 0)
                dma_v(pair, 1)
                mc = const_pool.tile([128, 2, QC], f16)
                nc.sync.dma_start(out=mc[:], in_=mconst)
                m2_t = const_pool.tile([128, 2, 128], f16)
                nc.sync.dma_start(out=m2_t[:], in_=m2)
            else:
                dma_k(pair, 0)
                dma_k(pair, 1)
                qp = qk_pool.tile([128, nqc * QC], f16, tag="qlo")
                nc.sync.dma_start(out=qp[:], in_=qt[pair])
                for c in range(nqc):
                    qseg[(pair, c)] = (qp, c * QC)
                dma_v(pair, 0)
                dma_v(pair, 1)

        # --- Flattened tile schedule per pair: start with the largest
        # chunk (warms the PE while later DMAs land), end with chunk 2 so
        # the tail still has full (non-diagonal) tiles to keep both exp
        # engines and the PE busy. Within a chunk the 4 diagonal tiles are
        # spread evenly among the full tiles (ctx accumulation order is
        # free) so consecutive tiles never form a single-engine exp run.
        tiles = []
        for pair in range(npair):
            for c in (nqc - 1, 1, 0, 2):
                nkt_c = (c + 1) * kt_per_qc
                diag0 = c * kt_per_qc
                fulls = list(range(diag0))
                diags = list(range(diag0, nkt_c))
                order = []
                fi = 0
                for di, dj in enumerate(diags):
                    order.append(dj)
                    take = round((di + 1) * len(fulls) / len(diags)) - fi
                    order.extend(fulls[fi : fi + take])
                    fi += take
                for pos, j in enumerate(order):
                    diag = j >= diag0
                    off = 128 * (j - diag0) if diag else 0
                    tiles.append(
                        (pair, c, j, diag, off, pos == 0, pos == nkt_c - 1)
                    )

        LEAD = 3  # score matmuls run LEAD tiles ahead (= sc pool bufs)
        sc_tiles = {}

        def emit_scores(idx):
            pair, c, j, diag, off, first, last = tiles[idx]
            ks = kseg[(pair, j // 8)]
            koff = (j % 8) * 128
            qs, qb = qseg[(pair, c)]
            sc = s_psum.tile([128, 2, QC], f32, tag="sc")
            nc.tensor.matmul(
                sc[:, 0, off:QC],
                lhsT=ks[0:D, koff : koff + 128],
                rhs=qs[0:D, qb + off : qb + QC],
                start=True,
                stop=True,
                tile_position=(0, 0),
            )
            nc.tensor.matmul(
                sc[:, 1, off:QC],
                lhsT=ks[D : 2 * D, koff : koff + 128],
                rhs=qs[D : 2 * D, qb + off : qb + QC],
                start=True,
                stop=True,
                tile_position=(64, 0),
            )
            sc_tiles[idx] = sc

        for k in range(min(LEAD, len(tiles))):
            emit_scores(k)
        # a few more warmups: the first ctx matmul waits on the first exp
        # (~1.3us away); keep the PE busy through that bubble so the HAM
        # window stays warm.
        for _ in range(WARM_POST):
            wmm = c_psum.tile([D + 1, QC], f32, tag="ctx")
            nc.tensor.matmul(
                wmm[:, 0:256],
                lhsT=warm_t[:, 0 : D + 1],
                rhs=warm_t[:],
                start=True,
                stop=True,
            )

        # Greedy engine balance for the non-diagonal exp tiles. Diagonal
        # tiles always go to VectorE (fused Schraudolph+mask); the cast
        # pair at each chunk end is split DVE/ACT.
        act_ns = 0.0
        dve_ns = 0.0

        ctx_a = ctx_b = None
        for idx in range(len(tiles)):
            pair, c, j, diag, off, first, last = tiles[idx]
            v_a = vseg[(pair, 0)]
            v_b = vseg[(pair, 1)]
            jj = j
            if first:
                ctx_a = c_psum.tile([D + 1, QC], f32, tag="ctx")
                ctx_b = c_psum.tile([D + 1, QC], f32, tag="ctx")
            sc = sc_tiles.pop(idx)
            ptile = pt_pool.tile([128, 2, QC], f16, tag="pt")
            n_el = 2 * (QC - off)
            cost_act = (208 + n_el) / 1.2
            cost_dve = (120 + n_el) / 0.96
            cost_stt = (151 + n_el) / 0.96
            cost_tt = (151 + 128) / 0.96  # 2x-mode fp16 mask multiply
            if diag:
                # Greedy: fused Schraudolph+mask on DVE vs exact exp on
                # ACT followed by a cheap DVE mask multiply.
                fin_a = max(act_ns, dve_ns + cost_stt)
                fin_b = max(act_ns + cost_act, dve_ns + cost_tt)
                if fin_a <= fin_b:
                    # Fused Schraudolph exp2 + causal mask on VectorE:
                    #   y = int16((s + B/A) * mtile); mtile = A or 0.
                    nc.vector.scalar_tensor_tensor(
                        out=ptile[:, :, off:QC].bitcast(i16),
                        in0=sc[:, :, off:QC],
                        scalar=float(SCH_BA),
                        in1=mc[:, :, 0 : QC - off],
                        op0=mybir.AluOpType.add,
                        op1=mybir.AluOpType.mult,
                    )
                    dve_ns += cost_stt
                else:
                    nc.scalar.activation(
                        out=ptile[:, :, off:QC],
                        in_=sc[:, :, off:QC],
                        func=mybir.ActivationFunctionType.Exp,
                        scale=SM_SCALE,
                    )
                    act_ns += cost_act
                    # zero P^T where q < k on the leading 128 columns
                    nc.vector.tensor_mul(
                        out=ptile[:, :, off : off + 128],
                        in0=ptile[:, :, off : off + 128],
                        in1=m2_t[:],
                    )
                    dve_ns += cost_tt
            else:
                if dve_ns + cost_dve < act_ns + cost_act:
                    # Schraudolph exp2 on VectorE: y=int16(s*A+B), bits
                    # read back as fp16 ~= exp(s*sm_scale).
                    nc.vector.tensor_scalar(
                        out=ptile[:, :, off:QC].bitcast(i16),
                        in0=sc[:, :, off:QC],
                        scalar1=float(SCH_A),
                        scalar2=float(SCH_B),
                        op0=mybir.AluOpType.mult,
                        op1=mybir.AluOpType.add,
                    )
                    dve_ns += cost_dve
                else:
                    nc.scalar.activation(
                        out=ptile[:, :, off:QC],
                        in_=sc[:, :, off:QC],
                        func=mybir.ActivationFunctionType.Exp,
                        scale=SM_SCALE,
                    )
                    act_ns += cost_act
            nc.tensor.matmul(
                ctx_a[:, off:QC],
                lhsT=v_a[:, jj, :],
                rhs=ptile[:, 0, off:QC],
                start=first,
                stop=last,
            )
            nc.tensor.matmul(
                ctx_b[:, off:QC],
                lhsT=v_b[:, jj, :],
                rhs=ptile[:, 1, off:QC],
                start=first,
                stop=last,
            )
            if idx + LEAD < len(tiles):
                emit_scores(idx + LEAD)
            if last:
                # split the two chunk-end casts across VectorE and ScalarE;
                # out DMAs on the sync HWDGE queue (all input DMAs were
                # issued long before the first chunk ends, so no contention
                # -- and HWDGE completes transfers much sooner than SWDGE,
                # which matters for the final chunk's drain).
                ia, ib = 2 * pair, 2 * pair + 1
                o_a = o_pool.tile([D + 1, QC], f16, tag="oa")
                nc.vector.tensor_copy(out=o_a[:], in_=ctx_a[:])
                nc.sync.dma_start(out=outT[ia, :, bass.ts(c, QC)], in_=o_a[:])
                o_b = o_pool.tile([D + 1, QC], f16, tag="ob")
                nc.scalar.copy(out=o_b[:], in_=ctx_b[:])
                nc.sync.dma_start(out=outT[ib, :, bass.ts(c, QC)], in_=o_b[:])
                dve_ns += (120 + QC) / 0.96
                act_ns += (208 + QC) / 1.2


def _make_mconst():
    # P^T layout is [k(partition), inst, q(col)]: the diagonal 128-block of
    # a diag tile sits at local cols 0..127 (keep iff q >= k -> upper
    # triangle); cols 128.. are fully unmasked.
    m = np.full((128, 2, QC), SCH_A16, np.float16)
    triu = np.triu(np.ones((128, 128), np.float16)) * np.float16(SCH_A16)
    m[:, 0, 0:128] = triu
    m[:, 1, 0:128] = triu
    return np.ascontiguousarray(m)


def _make_m2():
    triu = np.triu(np.ones((128, 128), np.float16))
    return np.ascontiguousarray(np.stack([triu, triu], axis=1))  # [128, 2, 128]


def _build_nc(seq=S, ni=NI):
    import concourse.tile as tile
    from concourse import bacc, mybir

    f16 = mybir.dt.float16
    nc = bacc.Bacc("TRN2")
    nkt = seq // 128
    qt = nc.dram_tensor("qt", [ni // 2, 2 * D, seq], f16, kind="ExternalInput")
    kt = nc.dram_tensor("kt", [ni // 2, 2 * D, seq], f16, kind="ExternalInput")
    v = nc.dram_tensor("v", [ni, 128, nkt, D + 1], f16, kind="ExternalInput")
    mconst = nc.dram_tensor("mconst", [128, 2, QC], f16, kind="ExternalInput")
    m2 = nc.dram_tensor("m2", [128, 2, 128], f16, kind="ExternalInput")
    outT = nc.dram_tensor("outT", [ni, D + 1, seq], f16, kind="ExternalOutput")
    with tile.TileContext(nc) as tc:
        _build_body(
            tc, outT, qt.ap(), kt.ap(), v.ap(), mconst.ap(), m2.ap(), seq, ni
        )
    nc.compile()
    return nc


def _get_nc():
    if "nc" not in _NC_CACHE:
        _NC_CACHE["nc"] = _build_nc()
    return _NC_CACHE["nc"]


def _numpy_fallback(query, key, value, attention_mask, causal_mask):
    b = query.shape[0]
    cm = np.broadcast_to(causal_mask, (b,) + causal_mask.shape[1:])
    am = attention_mask[:, None, None, :]
    mask = np.logical_and(cm, am)
    bias = np.where(mask, np.float32(0), np.finfo(np.float32).min).astype(np.float32)
    scale = np.float32(1.0 / np.sqrt(query.shape[-1]))
    scores = np.einsum("bqhd,bkhd->bhqk", query, key).astype(np.float32) * scale + bias
    scores = scores - scores.max(axis=-1, keepdims=True)
    p = np.exp(scores)
    p = p / p.sum(axis=-1, keepdims=True)
    ctx = np.einsum("bhqk,bkhd->bqhd", p.astype(np.float32), value)
    return ctx.reshape(ctx.shape[0], ctx.shape[1], -1).astype(np.float32)


def kernel(query, key, value, attention_mask, causal_mask):
    query = np.asarray(query, dtype=np.float32)
    key = np.asarray(key, dtype=np.float32)
    value = np.asarray(value, dtype=np.float32)
    attention_mask = np.asarray(attention_mask).astype(bool)
    causal_mask = np.asarray(causal_mask).astype(bool)

    tril = np.tril(np.ones((S, S), dtype=bool))
    if not (
        query.shape == (B, S, H, D)
        and attention_mask.all()
        and np.array_equal(causal_mask.reshape(S, S), tril)
    ):
        return _numpy_fallback(query, key, value, attention_mask, causal_mask)

    from concourse.bass_utils import run_bass_kernel_spmd

    nc = _get_nc()
    mconst = _make_mconst()
    m2 = _make_m2()
    nkt = S // 128
    in_maps = []
    for core in range(NCORES):
        insts = range(core * NI, (core + 1) * NI)
        qts = [query[i // H, :, i % H, :].T.astype(np.float16) for i in insts]
        kts = [key[i // H, :, i % H, :].T.astype(np.float16) for i in insts]
        qs = np.stack(
            [np.concatenate([qts[p], qts[p + 1]], axis=0) for p in range(0, NI, 2)]
        )
        ks = np.stack(
            [np.concatenate([kts[p], kts[p + 1]], axis=0) for p in range(0, NI, 2)]
        )
        # V_ext [S, 65] -> pre-permuted [128, nkt, 65] so DMA is contiguous
        vs = np.stack(
            [
                np.ascontiguousarray(
                    np.concatenate(
                        [value[i // H, :, i % H, :], np.ones((S, 1), np.float32)],
                        axis=1,
                    )
                    .astype(np.float16)
                    .reshape(nkt, 128, D + 1)
                    .transpose(1, 0, 2)
                )
                for i in insts
            ]
        )
        in_maps.append({"qt": qs, "kt": ks, "v": vs, "mconst": mconst, "m2": m2})

    res = run_bass_kernel_spmd(nc, in_maps, core_ids=list(range(NCORES)))
    _NC_CACHE["last_results"] = res

    out = np.empty((B, S, H, D), dtype=np.float32)
    for core in range(NCORES):
        o = np.asarray(res.results[core]["outT"], dtype=np.float32)  # [NI, 65, S]
        ctx = o[:, :D, :] / o[:, D : D + 1, :]
        for i_local, i in enumerate(range(core * NI, (core + 1) * NI)):
            out[i // H, :, i % H, :] = ctx[i_local].T
    return out.reshape(B, S, H * D)


# revision 25
# speedup vs baseline: 1.0047x; 1.0047x over previous
"""Causal multi-head attention (B=2, S=2048, H=16, D=64, fp32) on 8 trn2 cores.

Sharding: the 32 (batch, head) attention instances are split 4-per-core
(data parallel over B, tensor parallel over H) -- no collectives needed.

Device kernel (per core): instances are processed in PAIRS packed into the
128-deep PE array (K=64 each, row groups (0,0)/(64,0) stream concurrently).

Per pair, per query chunk of 512 (processed LARGEST-first so the PE's HAM
clock gate warms up early and stays warm):
  - scores transposed: S^T[k, q] via matmul(lhsT=K^T tile, rhs=Q^T chunk),
    both instances into one 2-bank PSUM tile.
  - P^T = exp(sm_scale * S^T):
      * non-diagonal k tiles: split between ScalarE ACTIVATE Exp and
        VectorE Schraudolph (tensor_scalar y = s*A + B written as int16,
        bits read back as fp16 ~ exp).
      * diagonal-region k tiles: VectorE scalar_tensor_tensor fusing the
        Schraudolph exp AND the causal mask in ONE op:
          y = int16((s + B/A) * mtile),  mtile = A where q >= k else 0
        masked lanes produce exactly 0.0 -> int16 0 -> fp16 +0.0.
  - ctx^T[d, q] = sum_k V_ext[k, d] P^T[k, q] accumulated in PSUM per
    instance; V_ext carries a ones column so row 64 is the softmax
    denominator. The [65, S] unnormalized ctx^T goes to HBM as fp16; the
    host divides by the denominator row and transposes into output layout.

Input DMAs are SEGMENTED (512-col chunks of Q^T/K^T, half-tiles of V) and
issued in first-use order so the first score matmul can start ~1.5us in,
instead of waiting for the full 3 MiB input fill. A short warmup matmul
burst covers the initial DMA window to keep the HAM clock gate warming.
"""

import numpy as np

B, S, H, D = 2, 2048, 16, 64
NCORES = 8
NI = (B * H) // NCORES  # attention instances per core
QC = 512  # query-chunk width (one PSUM bank of fp32)
SM_SCALE = 0.125  # 1/sqrt(D)
SCH_A = 1024.0 / np.log(2.0) * SM_SCALE  # Schraudolph multiplier
SCH_B = 15360.0 - 45.0  # fp16 exponent bias + centering constant
# fp16-exact Schraudolph multiplier used in the fused mask tile, and the
# matching pre-add constant so (s + SCH_BA) * SCH_A16 == s*SCH_A16 + SCH_B.
SCH_A16 = float(np.float16(SCH_A))
SCH_BA = SCH_B / SCH_A16
WARM = 14  # upfront warmup matmuls (cover the input-DMA window)
WARM_POST = 4  # extra warmups after the LEAD prefill (exp-latency ramp)

_NC_CACHE = {}


def _build_body(tc, outT, qt, kt, v, mconst, m2, seq, ni):
    import concourse.bass as bass
    from concourse import mybir

    nc = tc.nc
    f32 = mybir.dt.float32
    f16 = mybir.dt.float16
    i16 = mybir.dt.int16
    nkt = seq // 128  # key tiles per instance
    nqc = seq // QC  # query chunks per instance
    kt_per_qc = QC // 128
    assert ni % 2 == 0
    npair = ni // 2

    with (
        tc.tile_pool(name="const", bufs=1) as const_pool,
        tc.tile_pool(name="qk", bufs=npair * nqc) as qk_pool,
        tc.tile_pool(name="vp", bufs=2 * ni) as v_pool,
        tc.tile_pool(name="pt", bufs=6) as pt_pool,
        tc.tile_pool(name="ob", bufs=4) as o_pool,
        tc.tile_pool(name="sps", bufs=3, space="PSUM") as s_psum,
        tc.tile_pool(name="cps", bufs=2, space="PSUM") as c_psum,
    ):
        # --- HAM warmup: keep the PE busy during the first input DMA
        # segments so the clock gate starts warming before real work.
        warm_t = const_pool.tile([128, 256], f16)
        nc.vector.memset(warm_t[:], 0.0)
        for _ in range(WARM):
            wmm = s_psum.tile([128, 2, QC], f32, tag="sc")
            nc.tensor.matmul(
                wmm[:, :, 0:128],
                lhsT=warm_t[:, 0:128],
                rhs=warm_t[:],
                start=True,
                stop=True,
            )

        # --- Input DMAs: coarse (each dma_start costs ~650ns of serial
        # sync-engine issue time), but pair 0's first-needed data is split
        # so the first score matmul can start ~1.3us earlier: k in halves,
        # q's last chunk (the schedule starts at chunk 3) separated.
        kseg = {}  # (pair, half) -> [128, 2*QC] tile (k tiles 8h..8h+7)
        qseg = {}  # (pair, c) -> [128, QC] view into the pair's q tiles
        vseg = {}  # (pair, inst_in_pair) -> [128, nkt, D+1]
        mc = None

        def dma_k(pair, h):
            t = qk_pool.tile([128, 2 * QC], f16, tag="k")
            nc.sync.dma_start(out=t[:], in_=kt[pair][:, 2 * h * QC : 2 * (h + 1) * QC])
            kseg[(pair, h)] = t

        def dma_v(pair, iip):
            t = v_pool.tile([128, nkt, D + 1], f16, tag="v")
            nc.sync.dma_start(out=t[:], in_=v[2 * pair + iip])
            vseg[(pair, iip)] = t

        for pair in range(npair):
            if pair == 0:
                dma_k(pair, 0)
                qhi = qk_pool.tile([128, QC], f16, tag="q")
                nc.sync.dma_start(
                    out=qhi[:], in_=qt[pair][:, (nqc - 1) * QC : nqc * QC]
                )
                # mask constants next -- the first diagonal tile's exp
                # needs them, and anything queued behind the bulk q/k/v
                # transfers would arrive ~13us in and stall the ramp.
                mc = const_pool.tile([128, 2, QC], f16)
                nc.sync.dma_start(out=mc[:], in_=mconst)
                m2_t = const_pool.tile([128, 2, 128], f16)
                nc.sync.dma_start(out=m2_t[:], in_=m2)
                dma_k(pair, 1)
                qlo = qk_pool.tile([128, (nqc - 1) * QC], f16, tag="qlo")
                nc.sync.dma_start(out=qlo[:], in_=qt[pair][:, 0 : (nqc - 1) * QC])
                for c in range(nqc - 1):
                    qseg[(pair, c)] = (qlo, c * QC)
                qseg[(pair, nqc - 1)] = (qhi, 0)
                dma_v(pair, 0)
                dma_v(pair, 1)
            else:
                dma_k(pair, 0)
                dma_k(pair, 1)
                qp = qk_pool.tile([128, nqc * QC], f16, tag="qlo")
                nc.sync.dma_start(out=qp[:], in_=qt[pair])
                for c in range(nqc):
                    qseg[(pair, c)] = (qp, c * QC)
                dma_v(pair, 0)
                dma_v(pair, 1)

        # --- Flattened tile schedule per pair: start with the largest
        # chunk (warms the PE while later DMAs land), end with chunk 2 so
        # the tail still has full (non-diagonal) tiles to keep both exp
        # engines and the PE busy. Within a chunk the 4 diagonal tiles are
        # spread evenly among the full tiles (ctx accumulation order is
        # free) so consecutive tiles never form a single-engine exp run.
        tiles = []
        for pair in range(npair):
            for c in (nqc - 1, 1, 0, 2):
                nkt_c = (c + 1) * kt_per_qc
                diag0 = c * kt_per_qc
                fulls = list(range(diag0))
                diags = list(range(diag0, nkt_c))
                order = []
                fi = 0
                for di, dj in enumerate(diags):
                    take = round((di + 1) * len(fulls) / len(diags)) - fi
                    order.extend(fulls[fi : fi + take])
                    fi += take
                    order.append(dj)
                for pos, j in enumerate(order):
                    diag = j >= diag0
                    off = 128 * (j - diag0) if diag else 0
                    tiles.append(
                        (pair, c, j, diag, off, pos == 0, pos == nkt_c - 1)
                    )

        LEAD = 3  # score matmuls run LEAD tiles ahead (= sc pool bufs)
        sc_tiles = {}

        def emit_scores(idx):
            pair, c, j, diag, off, first, last = tiles[idx]
            ks = kseg[(pair, j // 8)]
            koff = (j % 8) * 128
            qs, qb = qseg[(pair, c)]
            sc = s_psum.tile([128, 2, QC], f32, tag="sc")
            nc.tensor.matmul(
                sc[:, 0, off:QC],
                lhsT=ks[0:D, koff : koff + 128],
                rhs=qs[0:D, qb + off : qb + QC],
                start=True,
                stop=True,
                tile_position=(0, 0),
            )
            nc.tensor.matmul(
                sc[:, 1, off:QC],
                lhsT=ks[D : 2 * D, koff : koff + 128],
                rhs=qs[D : 2 * D, qb + off : qb + QC],
                start=True,
                stop=True,
                tile_position=(64, 0),
            )
            sc_tiles[idx] = sc

        for k in range(min(LEAD, len(tiles))):
            emit_scores(k)
        # a few more warmups: the first ctx matmul waits on the first exp
        # (~1.3us away); keep the PE busy through that bubble so the HAM
        # window stays warm.
        for _ in range(WARM_POST):
            wmm = c_psum.tile([D + 1, QC], f32, tag="ctx")
            nc.tensor.matmul(
                wmm[:, 0:256],
                lhsT=warm_t[:, 0 : D + 1],
                rhs=warm_t[:],
                start=True,
                stop=True,
            )

        # Greedy engine balance for the non-diagonal exp tiles. Diagonal
        # tiles always go to VectorE (fused Schraudolph+mask); the cast
        # pair at each chunk end is split DVE/ACT.
        act_ns = 0.0
        dve_ns = 0.0

        ctx_a = ctx_b = None
        for idx in range(len(tiles)):
            pair, c, j, diag, off, first, last = tiles[idx]
            v_a = vseg[(pair, 0)]
            v_b = vseg[(pair, 1)]
            jj = j
            if first:
                ctx_a = c_psum.tile([D + 1, QC], f32, tag="ctx")
                ctx_b = c_psum.tile([D + 1, QC], f32, tag="ctx")
            sc = sc_tiles.pop(idx)
            ptile = pt_pool.tile([128, 2, QC], f16, tag="pt")
            n_el = 2 * (QC - off)
            cost_act = (208 + n_el) / 1.2
            cost_dve = (120 + n_el) / 0.96
            cost_stt = (151 + n_el) / 0.96
            cost_tt = (151 + 128) / 0.96  # 2x-mode fp16 mask multiply
            if diag:
                # Greedy: fused Schraudolph+mask on DVE vs exact exp on
                # ACT followed by a cheap DVE mask multiply.
                fin_a = max(act_ns, dve_ns + cost_stt)
                fin_b = max(act_ns + cost_act, dve_ns + cost_tt)
                if fin_a <= fin_b:
                    # Fused Schraudolph exp2 + causal mask on VectorE:
                    #   y = int16((s + B/A) * mtile); mtile = A or 0.
                    nc.vector.scalar_tensor_tensor(
                        out=ptile[:, :, off:QC].bitcast(i16),
                        in0=sc[:, :, off:QC],
                        scalar=float(SCH_BA),
                        in1=mc[:, :, 0 : QC - off],
                        op0=mybir.AluOpType.add,
                        op1=mybir.AluOpType.mult,
                    )
                    dve_ns += cost_stt
                else:
                    nc.scalar.activation(
                        out=ptile[:, :, off:QC],
                        in_=sc[:, :, off:QC],
                        func=mybir.ActivationFunctionType.Exp,
                        scale=SM_SCALE,
                    )
                    act_ns += cost_act
                    # zero P^T where q < k on the leading 128 columns
                    nc.vector.tensor_mul(
                        out=ptile[:, :, off : off + 128],
                        in0=ptile[:, :, off : off + 128],
                        in1=m2_t[:],
                    )
                    dve_ns += cost_tt
            else:
                if dve_ns + cost_dve < act_ns + cost_act:
                    # Schraudolph exp2 on VectorE: y=int16(s*A+B), bits
                    # read back as fp16 ~= exp(s*sm_scale).
                    nc.vector.tensor_scalar(
                        out=ptile[:, :, off:QC].bitcast(i16),
                        in0=sc[:, :, off:QC],
                        scalar1=float(SCH_A),
                        scalar2=float(SCH_B),
                        op0=mybir.AluOpType.mult,
                        op1=mybir.AluOpType.add,
                    )
                    dve_ns += cost_dve
                else:
                    nc.scalar.activation(
                        out=ptile[:, :, off:QC],
                        in_=sc[:, :, off:QC],
                        func=mybir.ActivationFunctionType.Exp,
                        scale=SM_SCALE,
                    )
                    act_ns += cost_act
            nc.tensor.matmul(
                ctx_a[:, off:QC],
                lhsT=v_a[:, jj, :],
                rhs=ptile[:, 0, off:QC],
                start=first,
                stop=last,
            )
            nc.tensor.matmul(
                ctx_b[:, off:QC],
                lhsT=v_b[:, jj, :],
                rhs=ptile[:, 1, off:QC],
                start=first,
                stop=last,
            )
            if idx + LEAD < len(tiles):
                emit_scores(idx + LEAD)
            if last:
                # split the two chunk-end casts across VectorE and ScalarE;
                # out DMAs on the sync HWDGE queue (all input DMAs were
                # issued long before the first chunk ends, so no contention
                # -- and HWDGE completes transfers much sooner than SWDGE,
                # which matters for the final chunk's drain).
                ia, ib = 2 * pair, 2 * pair + 1
                o_a = o_pool.tile([D + 1, QC], f16, tag="oa")
                nc.vector.tensor_copy(out=o_a[:], in_=ctx_a[:])
                nc.sync.dma_start(out=outT[ia, :, bass.ts(c, QC)], in_=o_a[:])
                o_b = o_pool.tile([D + 1, QC], f16, tag="ob")
                nc.scalar.copy(out=o_b[:], in_=ctx_b[:])
                nc.sync.dma_start(out=outT[ib, :, bass.ts(c, QC)], in_=o_b[:])
                dve_ns += (120 + QC) / 0.96
                act_ns += (208 + QC) / 1.2


def _make_mconst():
    # P^T layout is [k(partition), inst, q(col)]: the diagonal 128-block of
    # a diag tile sits at local cols 0..127 (keep iff q >= k -> upper
    # triangle); cols 128.. are fully unmasked.
    m = np.full((128, 2, QC), SCH_A16, np.float16)
    triu = np.triu(np.ones((128, 128), np.float16)) * np.float16(SCH_A16)
    m[:, 0, 0:128] = triu
    m[:, 1, 0:128] = triu
    return np.ascontiguousarray(m)


def _make_m2():
    triu = np.triu(np.ones((128, 128), np.float16))
    return np.ascontiguousarray(np.stack([triu, triu], axis=1))  # [128, 2, 128]


def _build_nc(seq=S, ni=NI):
    import concourse.tile as tile
    from concourse import bacc, mybir

    f16 = mybir.dt.float16
    nc = bacc.Bacc("TRN2")
    nkt = seq // 128
    qt = nc.dram_tensor("qt", [ni // 2, 2 * D, seq], f16, kind="ExternalInput")
    kt = nc.dram_tensor("kt", [ni // 2, 2 * D, seq], f16, kind="ExternalInput")
    v = nc.dram_tensor("v", [ni, 128, nkt, D + 1], f16, kind="ExternalInput")
    mconst = nc.dram_tensor("mconst", [128, 2, QC], f16, kind="ExternalInput")
    m2 = nc.dram_tensor("m2", [128, 2, 128], f16, kind="ExternalInput")
    outT = nc.dram_tensor("outT", [ni, D + 1, seq], f16, kind="ExternalOutput")
    with tile.TileContext(nc) as tc:
        _build_body(
            tc, outT, qt.ap(), kt.ap(), v.ap(), mconst.ap(), m2.ap(), seq, ni
        )
    nc.compile()
    return nc


def _get_nc():
    if "nc" not in _NC_CACHE:
        _NC_CACHE["nc"] = _build_nc()
    return _NC_CACHE["nc"]


def _numpy_fallback(query, key, value, attention_mask, causal_mask):
    b = query.shape[0]
    cm = np.broadcast_to(causal_mask, (b,) + causal_mask.shape[1:])
    am = attention_mask[:, None, None, :]
    mask = np.logical_and(cm, am)
    bias = np.where(mask, np.float32(0), np.finfo(np.float32).min).astype(np.float32)
    scale = np.float32(1.0 / np.sqrt(query.shape[-1]))
    scores = np.einsum("bqhd,bkhd->bhqk", query, key).astype(np.float32) * scale + bias
    scores = scores - scores.max(axis=-1, keepdims=True)
    p = np.exp(scores)
    p = p / p.sum(axis=-1, keepdims=True)
    ctx = np.einsum("bhqk,bkhd->bqhd", p.astype(np.float32), value)
    return ctx.reshape(ctx.shape[0], ctx.shape[1], -1).astype(np.float32)


def kernel(query, key, value, attention_mask, causal_mask):
    query = np.asarray(query, dtype=np.float32)
    key = np.asarray(key, dtype=np.float32)
    value = np.asarray(value, dtype=np.float32)
    attention_mask = np.asarray(attention_mask).astype(bool)
    causal_mask = np.asarray(causal_mask).astype(bool)

    tril = np.tril(np.ones((S, S), dtype=bool))
    if not (
        query.shape == (B, S, H, D)
        and attention_mask.all()
        and np.array_equal(causal_mask.reshape(S, S), tril)
    ):
        return _numpy_fallback(query, key, value, attention_mask, causal_mask)

    from concourse.bass_utils import run_bass_kernel_spmd

    nc = _get_nc()
    mconst = _make_mconst()
    m2 = _make_m2()
    nkt = S // 128
    in_maps = []
    for core in range(NCORES):
        insts = range(core * NI, (core + 1) * NI)
        qts = [query[i // H, :, i % H, :].T.astype(np.float16) for i in insts]
        kts = [key[i // H, :, i % H, :].T.astype(np.float16) for i in insts]
        qs = np.stack(
            [np.concatenate([qts[p], qts[p + 1]], axis=0) for p in range(0, NI, 2)]
        )
        ks = np.stack(
            [np.concatenate([kts[p], kts[p + 1]], axis=0) for p in range(0, NI, 2)]
        )
        # V_ext [S, 65] -> pre-permuted [128, nkt, 65] so DMA is contiguous
        vs = np.stack(
            [
                np.ascontiguousarray(
                    np.concatenate(
                        [value[i // H, :, i % H, :], np.ones((S, 1), np.float32)],
                        axis=1,
                    )
                    .astype(np.float16)
                    .reshape(nkt, 128, D + 1)
                    .transpose(1, 0, 2)
                )
                for i in insts
            ]
        )
        in_maps.append({"qt": qs, "kt": ks, "v": vs, "mconst": mconst, "m2": m2})

    res = run_bass_kernel_spmd(nc, in_maps, core_ids=list(range(NCORES)))
    _NC_CACHE["last_results"] = res

    out = np.empty((B, S, H, D), dtype=np.float32)
    for core in range(NCORES):
        o = np.asarray(res.results[core]["outT"], dtype=np.float32)  # [NI, 65, S]
        ctx = o[:, :D, :] / o[:, D : D + 1, :]
        for i_local, i in enumerate(range(core * NI, (core + 1) * NI)):
            out[i // H, :, i % H, :] = ctx[i_local].T
    return out.reshape(B, S, H * D)


# revision 28
# speedup vs baseline: 1.0353x; 1.0304x over previous
"""Causal multi-head attention (B=2, S=2048, H=16, D=64, fp32) on 8 trn2 cores.

Sharding: the 32 (batch, head) attention instances are split 4-per-core
(data parallel over B, tensor parallel over H) -- no collectives needed.

Device kernel (per core): instances are processed in PAIRS packed into the
128-deep PE array (K=64 each, row groups (0,0)/(64,0) stream concurrently).

Per pair, per query chunk of 512 (processed LARGEST-first so the PE's HAM
clock gate warms up early and stays warm):
  - scores transposed: S^T[k, q] via matmul(lhsT=K^T tile, rhs=Q^T chunk),
    both instances into one 2-bank PSUM tile.
  - P^T = exp(sm_scale * S^T):
      * non-diagonal k tiles: split between ScalarE ACTIVATE Exp and
        VectorE Schraudolph (tensor_scalar y = s*A + B written as int16,
        bits read back as fp16 ~ exp).
      * diagonal-region k tiles: VectorE scalar_tensor_tensor fusing the
        Schraudolph exp AND the causal mask in ONE op:
          y = int16((s + B/A) * mtile),  mtile = A where q >= k else 0
        masked lanes produce exactly 0.0 -> int16 0 -> fp16 +0.0.
  - ctx^T[d, q] = sum_k V_ext[k, d] P^T[k, q] accumulated in PSUM per
    instance; V_ext carries a ones column so row 64 is the softmax
    denominator. The [65, S] unnormalized ctx^T goes to HBM as fp16; the
    host divides by the denominator row and transposes into output layout.

Input DMAs are SEGMENTED (512-col chunks of Q^T/K^T, half-tiles of V) and
issued in first-use order so the first score matmul can start ~1.5us in,
instead of waiting for the full 3 MiB input fill. A short warmup matmul
burst covers the initial DMA window to keep the HAM clock gate warming.
"""

import numpy as np

B, S, H, D = 2, 2048, 16, 64
NCORES = 8
NI = (B * H) // NCORES  # attention instances per core
QC = 512  # query-chunk width (one PSUM bank of fp32)
SM_SCALE = 0.125  # 1/sqrt(D)
SCH_A = 1024.0 / np.log(2.0) * SM_SCALE  # Schraudolph multiplier
SCH_B = 15360.0 - 45.0  # fp16 exponent bias + centering constant
# fp16-exact Schraudolph multiplier used in the fused mask tile, and the
# matching pre-add constant so (s + SCH_BA) * SCH_A16 == s*SCH_A16 + SCH_B.
SCH_A16 = float(np.float16(SCH_A))
SCH_BA = SCH_B / SCH_A16
WARM = 10  # upfront warmup matmuls (cover the input-DMA window)
WARM_POST = 4  # extra warmups after the LEAD prefill (exp-latency ramp)

_NC_CACHE = {}


def _build_body(tc, outT, qt, kt, v, mconst, m2, seq, ni):
    import concourse.bass as bass
    from concourse import mybir

    nc = tc.nc
    f32 = mybir.dt.float32
    f16 = mybir.dt.float16
    i16 = mybir.dt.int16
    nkt = seq // 128  # key tiles per instance
    nqc = seq // QC  # query chunks per instance
    kt_per_qc = QC // 128
    assert ni % 2 == 0
    npair = ni // 2

    with (
        tc.tile_pool(name="const", bufs=1) as const_pool,
        tc.tile_pool(name="qk", bufs=npair * nqc) as qk_pool,
        tc.tile_pool(name="vp", bufs=2 * ni) as v_pool,
        tc.tile_pool(name="pt", bufs=6) as pt_pool,
        tc.tile_pool(name="ob", bufs=4) as o_pool,
        tc.tile_pool(name="sps", bufs=3, space="PSUM") as s_psum,
        tc.tile_pool(name="cps", bufs=2, space="PSUM") as c_psum,
    ):
        # --- HAM warmup: keep the PE busy during the first input DMA
        # segments so the clock gate starts warming before real work.
        warm_t = const_pool.tile([128, 256], f16)
        nc.vector.memset(warm_t[:], 0.0)
        for _ in range(WARM):
            wmm = s_psum.tile([128, 2, QC], f32, tag="sc")
            nc.tensor.matmul(
                wmm[:, :, 0:128],
                lhsT=warm_t[:, 0:128],
                rhs=warm_t[:],
                start=True,
                stop=True,
            )

        # --- Input DMAs: coarse (each dma_start costs ~650ns of serial
        # sync-engine issue time), issued strictly in first-use order so
        # the ramp never waits on data queued behind bulk transfers.
        # Pair 0's first tiles need: k tiles 0-3, q chunk 3, v_a, v_b, and
        # the mask constants -- those ~1.1 MB go first; everything else
        # lands well before its first use.
        kseg = {}  # (pair, j) -> ([128, *] tile, col offset of k tile j)
        qseg = {}  # (pair, c) -> ([128, *] tile, col offset of chunk c)
        vseg = {}  # (pair, inst_in_pair) -> [128, nkt, D+1]
        mc = None

        def dma_v(pair, iip):
            t = v_pool.tile([128, nkt, D + 1], f16, tag="v")
            nc.sync.dma_start(out=t[:], in_=v[2 * pair + iip])
            vseg[(pair, iip)] = t

        for pair in range(npair):
            if pair == 0:
                kq = qk_pool.tile([128, QC], f16, tag="k")
                nc.sync.dma_start(out=kq[:], in_=kt[pair][:, 0:QC])
                for j in range(kt_per_qc):
                    kseg[(pair, j)] = (kq, j * 128)
                qhi = qk_pool.tile([128, QC], f16, tag="q")
                nc.sync.dma_start(
                    out=qhi[:], in_=qt[pair][:, (nqc - 1) * QC : nqc * QC]
                )
                qseg[(pair, nqc - 1)] = (qhi, 0)
                dma_v(pair, 0)
                dma_v(pair, 1)
                mc = const_pool.tile([128, 2, QC], f16)
                nc.sync.dma_start(out=mc[:], in_=mconst)
                m2_t = const_pool.tile([128, 2, 128], f16)
                nc.sync.dma_start(out=m2_t[:], in_=m2)
                krest = qk_pool.tile([128, (nqc - 1) * QC], f16, tag="krest")
                nc.sync.dma_start(out=krest[:], in_=kt[pair][:, QC : nqc * QC])
                for j in range(kt_per_qc, nkt):
                    kseg[(pair, j)] = (krest, (j - kt_per_qc) * 128)
                qlo = qk_pool.tile([128, (nqc - 1) * QC], f16, tag="qlo")
                nc.sync.dma_start(out=qlo[:], in_=qt[pair][:, 0 : (nqc - 1) * QC])
                for c in range(nqc - 1):
                    qseg[(pair, c)] = (qlo, c * QC)
            else:
                kp = qk_pool.tile([128, nqc * QC], f16, tag="kfull")
                nc.sync.dma_start(out=kp[:], in_=kt[pair])
                for j in range(nkt):
                    kseg[(pair, j)] = (kp, j * 128)
                qp = qk_pool.tile([128, nqc * QC], f16, tag="qfull")
                nc.sync.dma_start(out=qp[:], in_=qt[pair])
                for c in range(nqc):
                    qseg[(pair, c)] = (qp, c * QC)
                dma_v(pair, 0)
                dma_v(pair, 1)

        # --- Flattened tile schedule per pair: start with the largest
        # chunk (warms the PE while later DMAs land), end with chunk 2 so
        # the tail still has full (non-diagonal) tiles to keep both exp
        # engines and the PE busy. Within a chunk the 4 diagonal tiles are
        # spread evenly among the full tiles (ctx accumulation order is
        # free) so consecutive tiles never form a single-engine exp run.
        tiles = []
        for pair in range(npair):
            for c in (nqc - 1, 1, 0, 2):
                nkt_c = (c + 1) * kt_per_qc
                diag0 = c * kt_per_qc
                fulls = list(range(diag0))
                diags = list(range(diag0, nkt_c))
                order = []
                fi = 0
                for di, dj in enumerate(diags):
                    take = round((di + 1) * len(fulls) / len(diags)) - fi
                    order.extend(fulls[fi : fi + take])
                    fi += take
                    order.append(dj)
                for pos, j in enumerate(order):
                    diag = j >= diag0
                    off = 128 * (j - diag0) if diag else 0
                    tiles.append(
                        (pair, c, j, diag, off, pos == 0, pos == nkt_c - 1)
                    )

        LEAD = 3  # score matmuls run LEAD tiles ahead (= sc pool bufs)
        sc_tiles = {}

        def emit_scores(idx):
            pair, c, j, diag, off, first, last = tiles[idx]
            ks, koff = kseg[(pair, j)]
            qs, qb = qseg[(pair, c)]
            sc = s_psum.tile([128, 2, QC], f32, tag="sc")
            nc.tensor.matmul(
                sc[:, 0, off:QC],
                lhsT=ks[0:D, koff : koff + 128],
                rhs=qs[0:D, qb + off : qb + QC],
                start=True,
                stop=True,
                tile_position=(0, 0),
            )
            nc.tensor.matmul(
                sc[:, 1, off:QC],
                lhsT=ks[D : 2 * D, koff : koff + 128],
                rhs=qs[D : 2 * D, qb + off : qb + QC],
                start=True,
                stop=True,
                tile_position=(64, 0),
            )
            sc_tiles[idx] = sc

        for k in range(min(LEAD, len(tiles))):
            emit_scores(k)
        # a few more warmups: the first ctx matmul waits on the first exp
        # (~1.3us away); keep the PE busy through that bubble so the HAM
        # window stays warm.
        for _ in range(WARM_POST):
            wmm = c_psum.tile([D + 1, QC], f32, tag="ctx")
            nc.tensor.matmul(
                wmm[:, 0:256],
                lhsT=warm_t[:, 0 : D + 1],
                rhs=warm_t[:],
                start=True,
                stop=True,
            )

        # Greedy engine balance for the non-diagonal exp tiles. Diagonal
        # tiles always go to VectorE (fused Schraudolph+mask); the cast
        # pair at each chunk end is split DVE/ACT.
        act_ns = 0.0
        dve_ns = 0.0

        ctx_a = ctx_b = None
        for idx in range(len(tiles)):
            pair, c, j, diag, off, first, last = tiles[idx]
            v_a = vseg[(pair, 0)]
            v_b = vseg[(pair, 1)]
            jj = j
            if first:
                ctx_a = c_psum.tile([D + 1, QC], f32, tag="ctx")
                ctx_b = c_psum.tile([D + 1, QC], f32, tag="ctx")
            sc = sc_tiles.pop(idx)
            ptile = pt_pool.tile([128, 2, QC], f16, tag="pt")
            n_el = 2 * (QC - off)
            cost_act = (208 + n_el) / 1.2
            cost_dve = (120 + n_el) / 0.96
            cost_stt = (151 + n_el) / 0.96
            cost_tt = (151 + 128) / 0.96  # 2x-mode fp16 mask multiply
            if diag:
                # Greedy: fused Schraudolph+mask on DVE vs exact exp on
                # ACT followed by a cheap DVE mask multiply.
                fin_a = max(act_ns, dve_ns + cost_stt)
                fin_b = max(act_ns + cost_act, dve_ns + cost_tt)
                if fin_a <= fin_b:
                    # Fused Schraudolph exp2 + causal mask on VectorE:
                    #   y = int16((s + B/A) * mtile); mtile = A or 0.
                    nc.vector.scalar_tensor_tensor(
                        out=ptile[:, :, off:QC].bitcast(i16),
                        in0=sc[:, :, off:QC],
                        scalar=float(SCH_BA),
                        in1=mc[:, :, 0 : QC - off],
                        op0=mybir.AluOpType.add,
                        op1=mybir.AluOpType.mult,
                    )
                    dve_ns += cost_stt
                else:
                    nc.scalar.activation(
                        out=ptile[:, :, off:QC],
                        in_=sc[:, :, off:QC],
                        func=mybir.ActivationFunctionType.Exp,
                        scale=SM_SCALE,
                    )
                    act_ns += cost_act
                    # zero P^T where q < k on the leading 128 columns
                    nc.vector.tensor_mul(
                        out=ptile[:, :, off : off + 128],
                        in0=ptile[:, :, off : off + 128],
                        in1=m2_t[:],
                    )
                    dve_ns += cost_tt
            else:
                if dve_ns + cost_dve < act_ns + cost_act:
                    # Schraudolph exp2 on VectorE: y=int16(s*A+B), bits
                    # read back as fp16 ~= exp(s*sm_scale).
                    nc.vector.tensor_scalar(
                        out=ptile[:, :, off:QC].bitcast(i16),
                        in0=sc[:, :, off:QC],
                        scalar1=float(SCH_A),
                        scalar2=float(SCH_B),
                        op0=mybir.AluOpType.mult,
                        op1=mybir.AluOpType.add,
                    )
                    dve_ns += cost_dve
                else:
                    nc.scalar.activation(
                        out=ptile[:, :, off:QC],
                        in_=sc[:, :, off:QC],
                        func=mybir.ActivationFunctionType.Exp,
                        scale=SM_SCALE,
                    )
                    act_ns += cost_act
            nc.tensor.matmul(
                ctx_a[:, off:QC],
                lhsT=v_a[:, jj, :],
                rhs=ptile[:, 0, off:QC],
                start=first,
                stop=last,
            )
            nc.tensor.matmul(
                ctx_b[:, off:QC],
                lhsT=v_b[:, jj, :],
                rhs=ptile[:, 1, off:QC],
                start=first,
                stop=last,
            )
            if idx + LEAD < len(tiles):
                emit_scores(idx + LEAD)
            if last:
                # split the two chunk-end casts across VectorE and ScalarE;
                # out DMAs on the sync HWDGE queue (all input DMAs were
                # issued long before the first chunk ends, so no contention
                # -- and HWDGE completes transfers much sooner than SWDGE,
                # which matters for the final chunk's drain).
                ia, ib = 2 * pair, 2 * pair + 1
                o_a = o_pool.tile([D + 1, QC], f16, tag="oa")
                nc.vector.tensor_copy(out=o_a[:], in_=ctx_a[:])
                nc.sync.dma_start(out=outT[ia, :, bass.ts(c, QC)], in_=o_a[:])
                o_b = o_pool.tile([D + 1, QC], f16, tag="ob")
                nc.scalar.copy(out=o_b[:], in_=ctx_b[:])
                nc.sync.dma_start(out=outT[ib, :, bass.ts(c, QC)], in_=o_b[:])
                dve_ns += (120 + QC) / 0.96
                act_ns += (208 + QC) / 1.2


def _make_mconst():
    # P^T layout is [k(partition), inst, q(col)]: the diagonal 128-block of
    # a diag tile sits at local cols 0..127 (keep iff q >= k -> upper
    # triangle); cols 128.. are fully unmasked.
    m = np.full((128, 2, QC), SCH_A16, np.float16)
    triu = np.triu(np.ones((128, 128), np.float16)) * np.float16(SCH_A16)
    m[:, 0, 0:128] = triu
    m[:, 1, 0:128] = triu
    return np.ascontiguousarray(m)


def _make_m2():
    triu = np.triu(np.ones((128, 128), np.float16))
    return np.ascontiguousarray(np.stack([triu, triu], axis=1))  # [128, 2, 128]


def _build_nc(seq=S, ni=NI):
    import concourse.tile as tile
    from concourse import bacc, mybir

    f16 = mybir.dt.float16
    nc = bacc.Bacc("TRN2")
    nkt = seq // 128
    qt = nc.dram_tensor("qt", [ni // 2, 2 * D, seq], f16, kind="ExternalInput")
    kt = nc.dram_tensor("kt", [ni // 2, 2 * D, seq], f16, kind="ExternalInput")
    v = nc.dram_tensor("v", [ni, 128, nkt, D + 1], f16, kind="ExternalInput")
    mconst = nc.dram_tensor("mconst", [128, 2, QC], f16, kind="ExternalInput")
    m2 = nc.dram_tensor("m2", [128, 2, 128], f16, kind="ExternalInput")
    outT = nc.dram_tensor("outT", [ni, D + 1, seq], f16, kind="ExternalOutput")
    with tile.TileContext(nc) as tc:
        _build_body(
            tc, outT, qt.ap(), kt.ap(), v.ap(), mconst.ap(), m2.ap(), seq, ni
        )
    nc.compile()
    return nc


def _get_nc():
    if "nc" not in _NC_CACHE:
        _NC_CACHE["nc"] = _build_nc()
    return _NC_CACHE["nc"]


def _numpy_fallback(query, key, value, attention_mask, causal_mask):
    b = query.shape[0]
    cm = np.broadcast_to(causal_mask, (b,) + causal_mask.shape[1:])
    am = attention_mask[:, None, None, :]
    mask = np.logical_and(cm, am)
    bias = np.where(mask, np.float32(0), np.finfo(np.float32).min).astype(np.float32)
    scale = np.float32(1.0 / np.sqrt(query.shape[-1]))
    scores = np.einsum("bqhd,bkhd->bhqk", query, key).astype(np.float32) * scale + bias
    scores = scores - scores.max(axis=-1, keepdims=True)
    p = np.exp(scores)
    p = p / p.sum(axis=-1, keepdims=True)
    ctx = np.einsum("bhqk,bkhd->bqhd", p.astype(np.float32), value)
    return ctx.reshape(ctx.shape[0], ctx.shape[1], -1).astype(np.float32)


def kernel(query, key, value, attention_mask, causal_mask):
    query = np.asarray(query, dtype=np.float32)
    key = np.asarray(key, dtype=np.float32)
    value = np.asarray(value, dtype=np.float32)
    attention_mask = np.asarray(attention_mask).astype(bool)
    causal_mask = np.asarray(causal_mask).astype(bool)

    tril = np.tril(np.ones((S, S), dtype=bool))
    if not (
        query.shape == (B, S, H, D)
        and attention_mask.all()
        and np.array_equal(causal_mask.reshape(S, S), tril)
    ):
        return _numpy_fallback(query, key, value, attention_mask, causal_mask)

    from concourse.bass_utils import run_bass_kernel_spmd

    nc = _get_nc()
    mconst = _make_mconst()
    m2 = _make_m2()
    nkt = S // 128
    in_maps = []
    for core in range(NCORES):
        insts = range(core * NI, (core + 1) * NI)
        qts = [query[i // H, :, i % H, :].T.astype(np.float16) for i in insts]
        kts = [key[i // H, :, i % H, :].T.astype(np.float16) for i in insts]
        qs = np.stack(
            [np.concatenate([qts[p], qts[p + 1]], axis=0) for p in range(0, NI, 2)]
        )
        ks = np.stack(
            [np.concatenate([kts[p], kts[p + 1]], axis=0) for p in range(0, NI, 2)]
        )
        # V_ext [S, 65] -> pre-permuted [128, nkt, 65] so DMA is contiguous
        vs = np.stack(
            [
                np.ascontiguousarray(
                    np.concatenate(
                        [value[i // H, :, i % H, :], np.ones((S, 1), np.float32)],
                        axis=1,
                    )
                    .astype(np.float16)
                    .reshape(nkt, 128, D + 1)
                    .transpose(1, 0, 2)
                )
                for i in insts
            ]
        )
        in_maps.append({"qt": qs, "kt": ks, "v": vs, "mconst": mconst, "m2": m2})

    res = run_bass_kernel_spmd(nc, in_maps, core_ids=list(range(NCORES)))
    _NC_CACHE["last_results"] = res

    out = np.empty((B, S, H, D), dtype=np.float32)
    for core in range(NCORES):
        o = np.asarray(res.results[core]["outT"], dtype=np.float32)  # [NI, 65, S]
        ctx = o[:, :D, :] / o[:, D : D + 1, :]
        for i_local, i in enumerate(range(core * NI, (core + 1) * NI)):
            out[i // H, :, i % H, :] = ctx[i_local].T
    return out.reshape(B, S, H * D)
